# revision 15
# baseline (speedup 1.0000x reference)
"""Trainium2 Bass kernel for ComplexDifferentialAttention (v4).

Sharding: 96 (head, q-tile-of-128) units over 8 cores; each core gets
8 q-tiles of one head (A) + 4 q-tiles of another head (B), so the SPMD
program is identical on every core: 3 batches of 4 q-tiles with
head-slot pattern (A, A, B).

v4 pipeline design:
- Startup DMAs split across 5 issue queues (sync/scalar/vector/gpsimd/
  tensor) in 512-1024 col pieces: a single DMA ring moves ~74GB/s, so
  parallel rings are required to reach the ~360GB/s HBM share.
- Scalar stream per batch b: squares(b,0) | sqrt(b,0) | ss2(b-1) |
  squares(b,1) | sqrt(b,1) | rms(b-1) | exp(b,0) exp(b,1) -- two ACT
  table loads per batch; Square rides in every table set.
- 2-step AV skew: av(b-1,*) matmuls run after sc(b,0); br1 PSUM is
  drained by t = av2*(-lam*inv2) + a1n (STT) and ss2 (scalar Square
  accum in the next sqrt window).
- s2 adds: DVE tensor_tensor f16 2x, pair-packed [128,1024] via
  2-level write APs from the squares. GpSimd does no bulk elementwise.
- Gating in bf16 at DVE 2x (tp PSUM drained to bf16 once), out-proj
  weights cast to bf16 on device so cr/ci can be bf16 stationary.
- Tail: av(2)+epilogue interleaved pair-major.
"""
import sys, os, math
sys.path.insert(0, '/opt/trn_rl_repo')
import numpy as np
from contextlib import ExitStack

import concourse.bacc as bacc
import concourse.tile as tile
from concourse import mybir
from concourse.bass_utils import run_bass_kernel_spmd
from concourse.masks import make_identity

F32 = mybir.dt.float32
F32R = mybir.dt.float32r
BF16 = mybir.dt.bfloat16
F16 = mybir.dt.float16
AF = mybir.ActivationFunctionType
OP = mybir.AluOpType

D = 128
S = 1024
H = 12
NCORES = 8
NB = 3          # batches per core, 4 units each
SCALE = 1.0 / math.sqrt(D)
LAMBDA_INIT = 0.8 - 0.6 * math.exp(-0.3)

# per-chunk square engine: 'S' = scalar Square direct from PSUM,
# 'D' = DVE copy(0.5x)+mult.  Tunable balance knob.
SQMODE_EVEN = ['S', 'D', 'S', 'D', 'S', 'D', 'S', 'D']
SQMODE_ODD = ['S', 'D', 'S', 'D', 'S', 'D', 'S', 'D']

# weight column map inside the packed wall tensor
WCOLS = {"wk_r": (0, 128), "wk_i": (128, 128), "wk_in": (256, 128),
         "wq_r": (384, 256), "wq_i": (640, 256), "wq_in": (896, 256),
         "wv1": (1152, 256), "wv2": (1408, 256),
         "wg_r": (1664, 128), "wg_i": (1792, 128), "wg_in": (1920, 128),
         "wo1": (2048, 256), "wo2": (2304, 256)}
WALLW = 2560

_prog_cache = {}


def _core_units(c):
    """Units for core c: list of (head, qtile). 8 of head A + 4 of head B."""
    k, odd = divmod(c, 2)
    hA = 3 * k + odd          # cores 2k -> 3k ; 2k+1 -> 3k+1
    hB = 3 * k + 2
    qoff = 0 if odd == 0 else 4
    return [(hA, q) for q in range(8)] + [(hB, qoff + q) for q in range(4)]


def _build_program():
    nc = bacc.Bacc("TRN2", target_bir_lowering=False, debug=False,
                   num_devices=NCORES)

    def din(name, shape, dt=F32R):
        return nc.dram_tensor(name, shape, dt, kind="ExternalInput").ap()

    # packed inputs (see _prep_inputs): per chunk ch: qtr|qti|pqr|pqi
    qpack = din("qpack", [128, 3 * 2048])
    # per head slot: ktr|kti|pkr|pki (4096) ; vtr|vti (2048)
    kpack = din("kpack", [128, 2 * 4096])
    vpack = din("vpack", [128, 2 * 2048])
    wall = din("wall", [128, WALLW])
    lamneg = din("lamneg", [128, 1], F32)
    lam2inv = din("lam2inv", [128, 1], F32)
    out_d = nc.dram_tensor("out", [12 * 128, 256], F32, kind="ExternalOutput").ap()

    with tile.TileContext(nc) as tc, ExitStack() as ctx:
        cst = ctx.enter_context(tc.tile_pool(name="cst", bufs=1))
        qpp = ctx.enter_context(tc.tile_pool(name="qpp", bufs=1))
        kpp = ctx.enter_context(tc.tile_pool(name="kpp", bufs=1))
        vpp = ctx.enter_context(tc.tile_pool(name="vpp", bufs=1))
        hot = ctx.enter_context(tc.tile_pool(name="hot", bufs=1))
        epi = ctx.enter_context(tc.tile_pool(name="epi", bufs=2))
        osb = ctx.enter_context(tc.tile_pool(name="osb", bufs=2))
        scp = ctx.enter_context(tc.tile_pool(name="scp", bufs=3, space="PSUM"))
        avp = ctx.enter_context(tc.tile_pool(name="avp", bufs=2, space="PSUM"))
        kinctx = ExitStack()
        kin = kinctx.enter_context(tc.tile_pool(name="kin", bufs=1))
        qinctx = ExitStack()
        qin = qinctx.enter_context(tc.tile_pool(name="qin", bufs=1))

        # ---- constants ----
        wallt = cst.tile([128, WALLW], F32R, name="wall", tag="wall")
        lam_t = cst.tile([128, 1], F32)
        l2i_t = cst.tile([128, 1], F32)
        ident = cst.tile([128, 128], F32)
        eps8 = cst.tile([128, 1], F32)
        eps5 = cst.tile([128, 1], F32)

        # ---- startup DMAs: parallel rings, priority pieces first ----
        kin_t = kin.tile([128, 4096], F32R, name="kin0", tag="kin0")
        qpt = qin.tile([128, 3 * 2048], F32R, name="qpt", tag="qpt")
        vin_t = kin.tile([128, 2048], F32R, name="vin0", tag="vin0")
        # 3 DMA rings (~80GB/s each): sync(SP), scalar(Act), gpsimd(Pool).
        # wave 1: wk+wq weights + ktr/kti split across rings
        nc.scalar.dma_start(wallt[:, 0:1152], wall[:, 0:1152])
        nc.sync.dma_start(kin_t[:, 0:512], kpack[:, 0:512])            # ktr c0
        nc.gpsimd.dma_start(kin_t[:, 512:1024], kpack[:, 512:1024])    # ktr c1
        nc.sync.dma_start(kin_t[:, 1024:1536], kpack[:, 1024:1536])    # kti c0
        nc.gpsimd.dma_start(kin_t[:, 1536:2048], kpack[:, 1536:2048])  # kti c1
        # wave 2: pe_k (kproj drains), q chunk 0
        nc.sync.dma_start(kin_t[:, 2048:3072], kpack[:, 2048:3072])    # pkr
        nc.gpsimd.dma_start(kin_t[:, 3072:4096], kpack[:, 3072:4096])  # pki
        nc.sync.dma_start(qpt[:, 0:1024], qpack[:, 0:1024])            # q ch0
        nc.gpsimd.dma_start(qpt[:, 1024:2048], qpack[:, 1024:2048])    # pe ch0
        # wave 3: rest of weights, q ch1/2, v slot 0
        nc.scalar.dma_start(wallt[:, 1152:WALLW], wall[:, 1152:WALLW])
        nc.scalar.dma_start(lam_t[:], lamneg[:])
        nc.scalar.dma_start(l2i_t[:], lam2inv[:])
        nc.sync.dma_start(qpt[:, 2048:4096], qpack[:, 2048:4096])      # ch1
        nc.gpsimd.dma_start(vin_t[:], vpack[:, 0:2048])
        nc.scalar.dma_start(qpt[:, 4096:6144], qpack[:, 4096:6144])    # ch2

        W = {nm: wallt[:, c:c + w] for nm, (c, w) in WCOLS.items()}
        make_identity(nc, ident[:])
        nc.vector.memset(eps8[:], 1e-8)
        nc.vector.memset(eps5[:], 1e-5)
        # bf16 copies of out-proj weights (bf16 stationary needs bf16 moving)
        wo1b = cst.tile([128, 256], BF16, name="wo1b", tag="wo1b")
        wo2b = cst.tile([128, 256], BF16, name="wo2b", tag="wo2b")
        nc.vector.tensor_copy(wo1b[:], W["wo1"])
        nc.vector.tensor_copy(wo2b[:], W["wo2"])

        def qsl(ch, part):
            # part: 0 qtr, 1 qti, 2 pqr, 3 pqi
            return qpt[:, ch * 2048 + part * 512: ch * 2048 + (part + 1) * 512]

        # ---- k projection (chunk-granular for startup overlap) ----
        kp = {}   # (hs, 'r'|'i'|'in') -> [d=128, k=1024] f32r

        def kv_kproj_chunk(hs, kt, part, chh):
            key = (hs, part)
            if key not in kp:
                kp[key] = kpp.tile([128, 1024], F32R, name=f"kp{hs}{part}",
                                   tag=f"kp{hs}{part}")
            t = kp[key]
            ktr = kt[:, 0:1024]; kti = kt[:, 1024:2048]
            pkr = kt[:, 2048:3072]; pki = kt[:, 3072:4096]
            cs = slice(chh * 512, (chh + 1) * 512)
            ps = scp.tile([128, 1024], F32, name="scps", tag="sc")
            if part == "r":
                nc.tensor.matmul(ps[:, 0:512], W["wk_r"], ktr[:, cs],
                                 start=True, stop=False)
                nc.tensor.matmul(ps[:, 0:512], W["wk_in"], kti[:, cs],
                                 start=False, stop=True)
                pe = pkr
            else:
                nc.tensor.matmul(ps[:, 0:512], W["wk_i"], ktr[:, cs],
                                 start=True, stop=False)
                nc.tensor.matmul(ps[:, 0:512], W["wk_r"], kti[:, cs],
                                 start=False, stop=True)
                pe = pki
            nc.vector.tensor_add(t[:, cs], ps[:, 0:512], pe[:, cs])

        def kv_kproj_neg(hs):
            tn = kpp.tile([128, 1024], F32R, name=f"kp{hs}in", tag=f"kp{hs}in")
            kp[(hs, "in")] = tn
            nc.vector.tensor_scalar_mul(tn[:], kp[(hs, "i")][:], -1.0)

        def kv_kproj(hs, kt):
            for chh in range(2):
                for part in ("r", "i"):
                    kv_kproj_chunk(hs, kt, part, chh)
            kv_kproj_neg(hs)

        # ---- v projection for a head slot ----
        vp = {}   # (hs, chunk) -> [k=128, 258] bf16  ([vp_r | vp_i | 1])

        def kv_vproj(hs, vt):
            vtr = vt[:, 0:1024]; vti = vt[:, 1024:2048]
            for chh in range(8):
                cs = slice(chh * 128, (chh + 1) * 128)
                ps = avp.tile([128, 258], F32, name="vps", tag="av")
                nc.tensor.matmul(ps[:, 0:256], vtr[:, cs], W["wv1"],
                                 start=True, stop=False)
                nc.tensor.matmul(ps[:, 0:256], vti[:, cs], W["wv2"],
                                 start=False, stop=True)
                vt_ = vpp.tile([128, 258], BF16, name=f"vp{hs}_{chh}",
                               tag=f"vp{hs}_{chh}")
                vp[(hs, chh)] = vt_
                if chh % 2 == 0:
                    nc.vector.tensor_copy(vt_[:, 0:256], ps[:, 0:256])
                else:
                    nc.scalar.copy(vt_[:, 0:256], ps[:, 0:256])
                nc.vector.memset(vt_[:, 256:258], 1.0)

        # ---- q projection chunk (chunk == batch) ----
        qp = {}
        for half in (0, 1):
            for part in ("r", "i"):
                qp[(half, part)] = qpp.tile([128, 1536], F32R,
                                            name=f"qp{half}{part}",
                                            tag=f"qp{half}{part}")

        def emit_qproj(ch):
            cs = slice(ch * 512, (ch + 1) * 512)
            for half in (0, 1):
                hs_ = slice(half * 128, (half + 1) * 128)
                for part in ("r", "i"):
                    t = qp[(half, part)]
                    ps = scp.tile([128, 1024], F32, name="scps", tag="sc")
                    if part == "r":
                        nc.tensor.matmul(ps[:, 0:512], W["wq_r"][:, hs_],
                                         qsl(ch, 0), start=True, stop=False)
                        nc.tensor.matmul(ps[:, 0:512], W["wq_in"][:, hs_],
                                         qsl(ch, 1), start=False, stop=True)
                        pe = qsl(ch, 2)
                    else:
                        nc.tensor.matmul(ps[:, 0:512], W["wq_i"][:, hs_],
                                         qsl(ch, 0), start=True, stop=False)
                        nc.tensor.matmul(ps[:, 0:512], W["wq_r"][:, hs_],
                                         qsl(ch, 1), start=False, stop=True)
                        pe = qsl(ch, 3)
                    nc.vector.tensor_add(t[:, cs], ps[:, 0:512], pe)

        # ---- gate projection chunk -> gT [128, 3072] bf16 (r | i) ----
        gT = qpp.tile([128, 3072], BF16, name="gT", tag="gT")
        gT3 = gT[:].rearrange("p (h c) -> p h c", h=2)

        def emit_gproj(ch):
            ps = scp.tile([128, 1024], F32, name="scps", tag="sc")
            nc.tensor.matmul(ps[:, 0:512], W["wg_r"], qsl(ch, 0),
                             start=True, stop=False)
            nc.tensor.matmul(ps[:, 0:512], W["wg_in"], qsl(ch, 1),
                             start=False, stop=True)
            nc.tensor.matmul(ps[:, 512:1024], W["wg_i"], qsl(ch, 0),
                             start=True, stop=False)
            nc.tensor.matmul(ps[:, 512:1024], W["wg_r"], qsl(ch, 1),
                             start=False, stop=True)
            ps3 = ps[:].rearrange("p (h c) -> p h c", h=2)
            nc.scalar.copy(gT3[:, :, ch * 512:(ch + 1) * 512], ps3)

        # =========== score pipeline ===========
        score_ps = {}   # (b, br) -> list of 8 psum tiles
        S2Q = {}        # (b, br) -> 2 s2 tiles [128,2048] f16
        et = {}         # (b, br, p) -> [128,2048] bf16

        def emit_scores(b, br):
            hs = 0 if b < 2 else 1
            qs = slice(b * 512, (b + 1) * 512)
            tiles = []
            for chh in range(8):
                cs = slice(chh * 128, (chh + 1) * 128)
                ps = scp.tile([128, 1024], F32, name="scps", tag="sc")
                nc.tensor.matmul(ps[:, 0:512], kp[(hs, "r")][:, cs],
                                 qp[(br, "r")][:, qs], start=True, stop=False)
                nc.tensor.matmul(ps[:, 0:512], kp[(hs, "i")][:, cs],
                                 qp[(br, "i")][:, qs], start=False, stop=True)
                nc.tensor.matmul(ps[:, 512:1024], kp[(hs, "r")][:, cs],
                                 qp[(br, "i")][:, qs], start=True, stop=False)
                nc.tensor.matmul(ps[:, 512:1024], kp[(hs, "in")][:, cs],
                                 qp[(br, "r")][:, qs], start=False, stop=True)
                tiles.append(ps)
            score_ps[(b, br)] = tiles

        def emit_sq(b, br):
            # squares of a chunk pair land in one sqp tile [128,2048]:
            # chunk 2j: sr->[0:512], si->[1024:1536]; 2j+1: sr->[512:1024],
            # si->[1536:2048]; one DVE f16 2x add [128,1024] per pair.
            mode = SQMODE_EVEN if br == 0 else SQMODE_ODD
            s2_tiles = [hot.tile([128, 2048], F16, name="s2", tag="s2",
                                 bufs=4) for _ in range(2)]
            for j in range(4):
                sqp = hot.tile([128, 2048], F16, name="sqp", tag="sqp",
                               bufs=3)
                sqp3 = sqp[:].rearrange("p (h c) -> p h c", h=2)
                for jj in (0, 1):
                    chh = 2 * j + jj
                    ps = score_ps[(b, br)][chh]
                    ps3 = ps[:].rearrange("p (h c) -> p h c", h=2)
                    dst = sqp3[:, :, jj * 512:(jj + 1) * 512]
                    if mode[chh] == 'S':
                        nc.scalar.activation(dst, ps3, AF.Square, scale=0.5)
                    else:
                        cp = hot.tile([128, 1024], F16, name="cp", tag="cp",
                                      bufs=3)
                        nc.vector.tensor_scalar_mul(cp[:], ps[:], 0.5)
                        cp3 = cp[:].rearrange("p (h c) -> p h c", h=2)
                        nc.vector.tensor_mul(dst, cp3, cp3)
                s2 = s2_tiles[j // 2]
                slot = slice((2 * j % 4) * 512, (2 * j % 4 + 2) * 512)
                nc.vector.tensor_add(s2[:, slot], sqp[:, 0:1024],
                                     sqp[:, 1024:2048])
            score_ps.pop((b, br))
            S2Q[(b, br)] = s2_tiles

        def emit_sqrt(b, br):
            for p in range(2):
                nc.scalar.activation(S2Q[(b, br)][p][:], S2Q[(b, br)][p][:],
                                     AF.Sqrt, bias=eps8[:])

        def emit_exp(b, br):
            for p in range(2):
                e = hot.tile([128, 2048], BF16, name="et", tag="et", bufs=6)
                nc.scalar.activation(e[:], S2Q[(b, br)][p][:], AF.Exp,
                                     scale=2.0 * SCALE)
                et[(b, br, p)] = e
            S2Q.pop((b, br))

        # =========== av + epilogue ===========
        EP = {}

        def emit_av_pass(b, br, p, d):
            # one uu-pair pass of one branch
            hs = 0 if b < 2 else 1
            uus = (2 * p, 2 * p + 1)
            avs = {}
            for uu in uus:
                avs[uu] = avp.tile([128, 258], F32, name=f"av{uu}", tag="av")
            for c in range(8):
                e = et[(b, br, c // 4)]
                base = (c % 4) * 512
                for uu in uus:
                    nc.tensor.matmul(
                        avs[uu][:],
                        e[:, base + uu * 128: base + (uu + 1) * 128],
                        vp[(hs, c)][:], start=(c == 0), stop=(c == 7))
            s, inv = (d["s1"], d["inv1"]) if br == 0 else (d["s2g"], d["inv2"])
            for uu in uus:
                nc.vector.tensor_copy(s[:, uu:uu + 1], avs[uu][:, 256:257])
            nc.vector.reciprocal(inv[:, 2 * p:2 * p + 2],
                                 s[:, 2 * p:2 * p + 2])
            if br == 0:
                for uu in uus:
                    nc.vector.tensor_scalar_mul(d["anp"][uu][:],
                                                avs[uu][:, 0:256],
                                                inv[:, uu:uu + 1])
            else:
                nc.vector.tensor_scalar_mul(d["s2v"][:, 2 * p:2 * p + 2],
                                            d["inv2"][:, 2 * p:2 * p + 2],
                                            lam_t[:, 0:1])
                for uu in uus:
                    # t = av2 * (-lam*inv2) + a1n
                    nc.vector.scalar_tensor_tensor(
                        d["t"][uu][:], avs[uu][:, 0:256],
                        d["s2v"][:, uu:uu + 1], d["anp"][uu][:],
                        op0=OP.mult, op1=OP.add)
                    # ss1_u = sum(a1n^2)
                    scr1 = epi.tile([128, 256], F32, name="scr1", tag="scr1",
                                    bufs=2)
                    nc.vector.scalar_tensor_tensor(
                        scr1[:], d["anp"][uu][:], 1.0, d["anp"][uu][:],
                        op0=OP.mult, op1=OP.mult,
                        accum_out=d["ss1"][:, uu:uu + 1])
                    # ss2_u = sum((lam*a2n)^2) = sum((a1n - t)^2);
                    # scaled by 1/lam^2 in the combine
                    dtl = epi.tile([128, 256], F32, name="dtl", tag="dtl",
                                   bufs=2)
                    nc.vector.tensor_sub(dtl[:], d["anp"][uu][:],
                                         d["t"][uu][:])
                    scr2 = epi.tile([128, 256], F32, name="scr2", tag="scr2",
                                    bufs=2)
                    nc.vector.scalar_tensor_tensor(
                        scr2[:], dtl[:], 1.0, dtl[:],
                        op0=OP.mult, op1=OP.mult,
                        accum_out=d["ss2"][:, uu:uu + 1])
                # per-pair combine: ss = ss1 + ss2/lam^2
                cs = slice(2 * p, 2 * p + 2)
                nc.vector.tensor_scalar_mul(d["i2"][:, cs], d["ss2"][:, cs],
                                            l2i_t[:, 0:1])
                nc.vector.tensor_add(d["ss"][:, cs], d["i2"][:, cs],
                                     d["ss1"][:, cs])

        def av_state(b):
            d = {}
            EP[b] = d
            d["anp"] = [epi.tile([128, 256], F32, name="anp", tag="anp",
                                 bufs=4) for _ in range(4)]
            d["t"] = [epi.tile([128, 256], F32, name="tt", tag="tt", bufs=4)
                      for _ in range(4)]
            for nm in ("s1", "inv1", "s2g", "inv2", "s2v", "ss1", "ss2",
                       "ss", "i2"):
                d[nm] = epi.tile([128, 4], F32, name=nm, tag=nm)
            return d

        def emit_av_batch(b):
            d = av_state(b)
            for br in (0, 1):
                for p in (0, 1):
                    emit_av_pass(b, br, p, d)

        def emit_rms(b, p):
            d = EP[b]
            if p == 0:
                d["rms"] = epi.tile([128, 4], F32, name="rms", tag="rms")
                d["rinv"] = epi.tile([128, 4], F32, name="rinv", tag="rinv")
            cs = slice(2 * p, 2 * p + 2)
            nc.scalar.activation(d["rms"][:, cs], d["ss"][:, cs], AF.Sqrt,
                                 bias=eps5[:], scale=1.0 / 256.0)

        def emit_ep_pair(b, pi):
            # rinv, transpose, bf16 gating at DVE 2x, out-proj, out copies
            d = EP[b]
            cs = slice(2 * pi, 2 * pi + 2)
            nc.vector.reciprocal(d["rinv"][:, cs], d["rms"][:, cs])
            tp = scp.tile([128, 1024], F32, name="scps", tag="sc")
            for j in range(2):
                u = 2 * pi + j
                nc.tensor.transpose(tp[:, j * 128:(j + 1) * 128],
                                    d["t"][u][:, 0:128], ident[:])
                nc.tensor.transpose(tp[:, 256 + j * 128: 256 + (j + 1) * 128],
                                    d["t"][u][:, 128:256], ident[:])
            tps = epi.tile([128, 512], BF16, name="tps", tag="tps", bufs=2)
            nc.vector.tensor_copy(tps[:], tp[:, 0:512])
            gr = gT[:, b * 512 + pi * 256: b * 512 + (pi + 1) * 256]
            gi = gT[:, 1536 + b * 512 + pi * 256:
                    1536 + b * 512 + (pi + 1) * 256]
            m1 = epi.tile([128, 256], BF16, name="m1", tag="m1", bufs=2)
            m2 = epi.tile([128, 256], BF16, name="m2", tag="m2", bufs=2)
            m3 = epi.tile([128, 256], BF16, name="m3", tag="m3", bufs=2)
            m4 = epi.tile([128, 256], BF16, name="m4", tag="m4", bufs=2)
            nc.vector.tensor_mul(m1[:], gr, tps[:, 0:256])
            nc.vector.tensor_mul(m2[:], gi, tps[:, 256:512])
            nc.vector.tensor_mul(m3[:], gi, tps[:, 0:256])
            nc.vector.tensor_mul(m4[:], gr, tps[:, 256:512])
            cr = epi.tile([128, 256], BF16, name="cr", tag="cr", bufs=2)
            ci = epi.tile([128, 256], BF16, name="ci", tag="ci", bufs=2)
            nc.gpsimd.tensor_sub(cr[:], m1[:], m2[:])
            nc.gpsimd.tensor_add(ci[:], m3[:], m4[:])
            po = scp.tile([128, 1024], F32, name="scps", tag="sc")
            for j in range(2):
                pos = slice(j * 256, (j + 1) * 256)
                nc.tensor.matmul(po[:, pos], cr[:, j * 128:(j + 1) * 128],
                                 wo1b[:], start=True, stop=False)
                nc.tensor.matmul(po[:, pos], ci[:, j * 128:(j + 1) * 128],
                                 wo2b[:], start=False, stop=True)
            for j in range(2):
                u = 2 * pi + j
                iu = b * 4 + u
                ot = osb.tile([128, 256], F32, name="ot", tag="ot")
                nc.scalar.mul(ot[:], po[:, j * 256:(j + 1) * 256],
                              d["rinv"][:, u:u + 1])
                nc.sync.dma_start(out_d[iu * 128:(iu + 1) * 128, :], ot[:])

        # ---- emission sequence ----
        # prep: kproj chunk 0 asap, then drains, qproj ch0, scores
        for part in ("r", "i"):
            kv_kproj_chunk(0, kin_t, part, 0)
        for part in ("r", "i"):
            kv_kproj_chunk(0, kin_t, part, 1)
        kv_kproj_neg(0)
        emit_qproj(0)
        emit_scores(0, 0); emit_sq(0, 0)
        emit_sqrt(0, 0)
        emit_scores(0, 1); emit_sq(0, 1)
        emit_sqrt(0, 1)
        emit_exp(0, 0); emit_exp(0, 1)
        emit_gproj(0)
        kv_vproj(0, vin_t)
        emit_qproj(1)

        # head slot 1 k-side DMAs (sync: before any data-dependent issues)
        kin1 = kin.tile([128, 4096], F32R, name="kin1", tag="kin0")
        nc.sync.dma_start(kin1[:, 0:2048], kpack[:, 4096:6144])
        nc.sync.dma_start(kin1[:, 2048:4096], kpack[:, 6144:8192])
        vin1 = kin.tile([128, 2048], F32R, name="vin1", tag="vin0")
        nc.gpsimd.dma_start(vin1[:], vpack[:, 2048:4096])

        emit_scores(1, 0); emit_sq(1, 0)
        emit_sqrt(1, 0)
        emit_av_batch(0)
        emit_scores(1, 1); emit_sq(1, 1)
        emit_sqrt(1, 1)
        emit_rms(0, 0); emit_rms(0, 1)
        emit_exp(1, 0); emit_exp(1, 1)
        emit_ep_pair(0, 0); emit_ep_pair(0, 1)

        kv_kproj(1, kin1)
        kv_vproj(1, vin1)
        emit_gproj(1)
        emit_qproj(2)
        emit_gproj(2)
        qinctx.close()

        emit_scores(2, 0); emit_sq(2, 0)
        emit_sqrt(2, 0)
        emit_av_batch(1)
        emit_scores(2, 1); emit_sq(2, 1)
        emit_sqrt(2, 1)
        emit_rms(1, 0); emit_rms(1, 1)
        emit_exp(2, 0); emit_exp(2, 1)
        emit_ep_pair(1, 0); emit_ep_pair(1, 1)
        kinctx.close()

        # tail: pair-major av(2) + epilogue interleave
        d2 = av_state(2)
        emit_av_pass(2, 0, 0, d2)
        emit_av_pass(2, 1, 0, d2)
        emit_rms(2, 0)
        emit_ep_pair(2, 0)
        emit_av_pass(2, 0, 1, d2)
        emit_av_pass(2, 1, 1, d2)
        emit_rms(2, 1)
        emit_ep_pair(2, 1)

    nc.compile()
    return nc


def _get_program():
    if "nc" not in _prog_cache:
        _prog_cache["nc"] = _build_program()
    return _prog_cache["nc"]


def _prep_inputs(inputs):
    f = {k: np.asarray(v, dtype=np.float32) for k, v in inputs.items()}
    lam1 = np.float32(np.exp(np.float32(np.sum(f["lq1"] * f["lk1"]))))
    lam2 = np.float32(np.exp(np.float32(np.sum(f["lq2"] * f["lk2"]))))
    x = np.float32(lam1 - lam2 + np.float32(LAMBDA_INIT))
    lam = np.float32(1.0 / (1.0 + np.exp(-x)))

    wq_rT = f["qw_r"].T          # [128, 256]
    wq_iT = f["qw_i"].T
    wk_rT = f["kw_r"].T          # [128, 128]
    wk_iT = f["kw_i"].T
    vw_rT = f["vw_r"].T; vw_iT = f["vw_i"].T
    wv1 = np.concatenate([vw_rT, vw_iT], 1)
    wv2 = np.concatenate([-vw_iT, vw_rT], 1)
    wg_rT = f["gw_r"].T; wg_iT = f["gw_i"].T
    ow_rT = f["ow_r"].T; ow_iT = f["ow_i"].T
    wo1 = np.concatenate([ow_rT, ow_iT], 1)
    wo2 = np.concatenate([-ow_iT, ow_rT], 1)
    wmap = {"wk_r": wk_rT, "wk_i": wk_iT, "wk_in": -wk_iT,
            "wq_r": wq_rT, "wq_i": wq_iT, "wq_in": -wq_iT,
            "wv1": wv1, "wv2": wv2,
            "wg_r": wg_rT, "wg_i": wg_iT, "wg_in": -wg_iT,
            "wo1": wo1, "wo2": wo2}
    wall = np.zeros((128, WALLW), np.float32)
    for nm, (c, w) in WCOLS.items():
        wall[:, c:c + w] = wmap[nm]
    shared = {
        "wall": wall,
        "lamneg": np.full((128, 1), -lam, np.float32),
        "lam2inv": np.full((128, 1), 1.0 / (lam * lam), np.float32),
    }

    in_maps = []
    for c in range(NCORES):
        units = _core_units(c)
        heads = [units[0][0], units[8][0]]
        m = dict(shared)

        def pack_q(t):
            cols = [t[0, h, q * 128:(q + 1) * 128, :].T for (h, q) in units]
            return np.concatenate(cols, 1)
        qtr = pack_q(f["q_r"]); qti = pack_q(f["q_i"])
        pqr = pack_q(f["pe_q_r"]); pqi = pack_q(f["pe_q_i"])
        m["qpack"] = np.ascontiguousarray(np.concatenate(
            [np.concatenate([qtr[:, ch * 512:(ch + 1) * 512],
                             qti[:, ch * 512:(ch + 1) * 512],
                             pqr[:, ch * 512:(ch + 1) * 512],
                             pqi[:, ch * 512:(ch + 1) * 512]], 1)
             for ch in range(3)], 1))

        kk = []
        vv = []
        for h in heads:
            kk.append(np.concatenate(
                [f["k_r"][0, h].T, f["k_i"][0, h].T,
                 f["pe_k_r"][0, h].T, f["pe_k_i"][0, h].T], 1))
            vv.append(np.concatenate([f["v_r"][0, h].T, f["v_i"][0, h].T], 1))
        m["kpack"] = np.ascontiguousarray(np.concatenate(kk, 1))
        m["vpack"] = np.ascontiguousarray(np.concatenate(vv, 1))
        in_maps.append(m)
    return in_maps


def _unpack(results):
    out_r = np.zeros((1, H, S, D), np.float32)
    out_i = np.zeros((1, H, S, D), np.float32)
    for c in range(NCORES):
        o = results[c]["out"]
        for u, (h, q) in enumerate(_core_units(c)):
            blk = o[u * 128:(u + 1) * 128]
            out_r[0, h, q * 128:(q + 1) * 128, :] = blk[:, 0:128]
            out_i[0, h, q * 128:(q + 1) * 128, :] = blk[:, 128:256]
    return out_r, out_i


def _run(inputs, trace=False, tmpdir=None):
    nc = _get_program()
    in_maps = _prep_inputs(inputs)
    res = run_bass_kernel_spmd(nc, in_maps, list(range(NCORES)), trace=trace,
                               tmpdir=tmpdir)
    return _unpack(res.results), res


def kernel(**inputs):
    (out_r, out_i), _ = _run(inputs, trace=False)
    return out_r, out_i


# revision 22
# speedup vs baseline: 1.0629x; 1.0629x over previous
"""Trainium2 Bass kernel for ComplexDifferentialAttention (v4).

Sharding: 96 (head, q-tile-of-128) units over 8 cores; each core gets
8 q-tiles of one head (A) + 4 q-tiles of another head (B), so the SPMD
program is identical on every core: 3 batches of 4 q-tiles with
head-slot pattern (A, A, B).

v4 pipeline design:
- Startup DMAs split across 5 issue queues (sync/scalar/vector/gpsimd/
  tensor) in 512-1024 col pieces: a single DMA ring moves ~74GB/s, so
  parallel rings are required to reach the ~360GB/s HBM share.
- Scalar stream per batch b: squares(b,0) | sqrt(b,0) | ss2(b-1) |
  squares(b,1) | sqrt(b,1) | rms(b-1) | exp(b,0) exp(b,1) -- two ACT
  table loads per batch; Square rides in every table set.
- 2-step AV skew: av(b-1,*) matmuls run after sc(b,0); br1 PSUM is
  drained by t = av2*(-lam*inv2) + a1n (STT) and ss2 (scalar Square
  accum in the next sqrt window).
- s2 adds: DVE tensor_tensor f16 2x, pair-packed [128,1024] via
  2-level write APs from the squares. GpSimd does no bulk elementwise.
- Gating in bf16 at DVE 2x (tp PSUM drained to bf16 once), out-proj
  weights cast to bf16 on device so cr/ci can be bf16 stationary.
- Tail: av(2)+epilogue interleaved pair-major.
"""
import sys, os, math
sys.path.insert(0, '/opt/trn_rl_repo')
import numpy as np
from contextlib import ExitStack

import concourse.bacc as bacc
import concourse.tile as tile
from concourse import mybir
from concourse.bass_utils import run_bass_kernel_spmd
from concourse.masks import make_identity

F32 = mybir.dt.float32
F32R = mybir.dt.float32r
BF16 = mybir.dt.bfloat16
F16 = mybir.dt.float16
AF = mybir.ActivationFunctionType
OP = mybir.AluOpType

D = 128
S = 1024
H = 12
NCORES = 8
NB = 3          # batches per core, 4 units each
SCALE = 1.0 / math.sqrt(D)
LAMBDA_INIT = 0.8 - 0.6 * math.exp(-0.3)

# per-chunk square engine: 'S' = scalar Square direct from PSUM,
# 'D' = DVE copy(0.5x)+mult.  Tunable balance knob.
SQMODE_EVEN = ['S', 'D', 'S', 'D', 'S', 'D', 'S', 'S']
SQMODE_ODD = ['S', 'D', 'S', 'D', 'S', 'D', 'S', 'D']

# weight column map inside the packed wall tensor
WCOLS = {"wk_r": (0, 128), "wk_i": (128, 128), "wk_in": (256, 128),
         "wq_r": (384, 256), "wq_i": (640, 256), "wq_in": (896, 256),
         "wv1": (1152, 256), "wv2": (1408, 256),
         "wg_r": (1664, 128), "wg_i": (1792, 128), "wg_in": (1920, 128),
         "wo1": (2048, 256), "wo2": (2304, 256)}
WALLW = 2560

_prog_cache = {}


def _core_units(c):
    """Units for core c: list of (head, qtile). 8 of head A + 4 of head B."""
    k, odd = divmod(c, 2)
    hA = 3 * k + odd          # cores 2k -> 3k ; 2k+1 -> 3k+1
    hB = 3 * k + 2
    qoff = 0 if odd == 0 else 4
    return [(hA, q) for q in range(8)] + [(hB, qoff + q) for q in range(4)]


def _build_program():
    nc = bacc.Bacc("TRN2", target_bir_lowering=False, debug=False,
                   num_devices=NCORES)

    def din(name, shape, dt=F32R):
        return nc.dram_tensor(name, shape, dt, kind="ExternalInput").ap()

    # packed inputs (see _prep_inputs): per chunk ch: qtr|qti|pqr|pqi
    qpack = din("qpack", [128, 3 * 2048])
    # per head slot: ktr|kti|pkr|pki (4096) ; vtr|vti (2048)
    kpack = din("kpack", [128, 2 * 4096])
    vpack = din("vpack", [128, 2 * 2048])
    wall = din("wall", [128, WALLW])
    lamneg = din("lamneg", [128, 1], F32)
    lam2inv = din("lam2inv", [128, 1], F32)
    out_d = nc.dram_tensor("out", [12 * 128, 256], F32, kind="ExternalOutput").ap()

    with tile.TileContext(nc) as tc, ExitStack() as ctx:
        cst = ctx.enter_context(tc.tile_pool(name="cst", bufs=1))
        qpp = ctx.enter_context(tc.tile_pool(name="qpp", bufs=1))
        kpp = ctx.enter_context(tc.tile_pool(name="kpp", bufs=1))
        vpp = ctx.enter_context(tc.tile_pool(name="vpp", bufs=1))
        hot = ctx.enter_context(tc.tile_pool(name="hot", bufs=1))
        epi = ctx.enter_context(tc.tile_pool(name="epi", bufs=2))
        osb = ctx.enter_context(tc.tile_pool(name="osb", bufs=2))
        scp = ctx.enter_context(tc.tile_pool(name="scp", bufs=3, space="PSUM"))
        avp = ctx.enter_context(tc.tile_pool(name="avp", bufs=2, space="PSUM"))
        kinctx = ExitStack()
        kin = kinctx.enter_context(tc.tile_pool(name="kin", bufs=1))
        qinctx = ExitStack()
        qin = qinctx.enter_context(tc.tile_pool(name="qin", bufs=1))

        # ---- constants ----
        wallt = cst.tile([128, WALLW], F32R, name="wall", tag="wall")
        lam_t = cst.tile([128, 1], F32)
        l2i_t = cst.tile([128, 1], F32)
        ident = cst.tile([128, 128], F32)
        eps8 = cst.tile([128, 1], F32)
        eps5 = cst.tile([128, 1], F32)

        # ---- startup DMAs: parallel rings, priority pieces first ----
        kin_t = kin.tile([128, 4096], F32R, name="kin0", tag="kin0")
        qpt = qin.tile([128, 3 * 2048], F32R, name="qpt", tag="qpt")
        vin_t = kin.tile([128, 2048], F32R, name="vin0", tag="vin0")
        # 3 DMA rings (~80GB/s each): sync(SP), scalar(Act), gpsimd(Pool).
        # ~3.6MB is score-critical; balance it ~1.2MB per ring.
        nc.scalar.dma_start(wallt[:, 0:1152], wall[:, 0:1152])
        nc.sync.dma_start(kin_t[:, 0:512], kpack[:, 0:512])            # ktr c0
        nc.gpsimd.dma_start(kin_t[:, 512:1024], kpack[:, 512:1024])    # ktr c1
        nc.sync.dma_start(kin_t[:, 1024:1536], kpack[:, 1024:1536])    # kti c0
        nc.gpsimd.dma_start(kin_t[:, 1536:2048], kpack[:, 1536:2048])  # kti c1
        nc.scalar.dma_start(qpt[:, 0:1024], qpack[:, 0:1024])          # q ch0
        nc.sync.dma_start(kin_t[:, 2048:3072], kpack[:, 2048:3072])    # pkr
        nc.gpsimd.dma_start(kin_t[:, 3072:4096], kpack[:, 3072:4096])  # pki
        nc.sync.dma_start(qpt[:, 1024:1536], qpack[:, 1024:1536])      # pe r c0
        nc.gpsimd.dma_start(qpt[:, 1536:2048], qpack[:, 1536:2048])    # pe i c0
        # non-critical remainder
        nc.scalar.dma_start(wallt[:, 1152:WALLW], wall[:, 1152:WALLW])
        nc.scalar.dma_start(lam_t[:], lamneg[:])
        nc.scalar.dma_start(l2i_t[:], lam2inv[:])
        nc.sync.dma_start(qpt[:, 2048:4096], qpack[:, 2048:4096])      # ch1
        nc.gpsimd.dma_start(vin_t[:], vpack[:, 0:2048])
        nc.scalar.dma_start(qpt[:, 4096:6144], qpack[:, 4096:6144])    # ch2

        W = {nm: wallt[:, c:c + w] for nm, (c, w) in WCOLS.items()}
        make_identity(nc, ident[:])
        nc.vector.memset(eps8[:], 1e-8)
        nc.vector.memset(eps5[:], 1e-5)
        # bf16 copies of out-proj weights (bf16 stationary needs bf16 moving)
        wo1b = cst.tile([128, 256], BF16, name="wo1b", tag="wo1b")
        wo2b = cst.tile([128, 256], BF16, name="wo2b", tag="wo2b")
        nc.vector.tensor_copy(wo1b[:], W["wo1"])
        nc.vector.tensor_copy(wo2b[:], W["wo2"])

        def qsl(ch, part):
            # part: 0 qtr, 1 qti, 2 pqr, 3 pqi
            return qpt[:, ch * 2048 + part * 512: ch * 2048 + (part + 1) * 512]

        # ---- k projection (chunk-granular for startup overlap) ----
        kp = {}   # (hs, 'r'|'i'|'in') -> [d=128, k=1024] f32r

        def kv_kproj_chunk(hs, kt, part, chh):
            key = (hs, part)
            if key not in kp:
                kp[key] = kpp.tile([128, 1024], F32R, name=f"kp{hs}{part}",
                                   tag=f"kp{hs}{part}")
            t = kp[key]
            ktr = kt[:, 0:1024]; kti = kt[:, 1024:2048]
            pkr = kt[:, 2048:3072]; pki = kt[:, 3072:4096]
            cs = slice(chh * 512, (chh + 1) * 512)
            ps = scp.tile([128, 1024], F32, name="scps", tag="sc")
            if part == "r":
                nc.tensor.matmul(ps[:, 0:512], W["wk_r"], ktr[:, cs],
                                 start=True, stop=False)
                nc.tensor.matmul(ps[:, 0:512], W["wk_in"], kti[:, cs],
                                 start=False, stop=True)
                pe = pkr
            else:
                nc.tensor.matmul(ps[:, 0:512], W["wk_i"], ktr[:, cs],
                                 start=True, stop=False)
                nc.tensor.matmul(ps[:, 0:512], W["wk_r"], kti[:, cs],
                                 start=False, stop=True)
                pe = pki
            nc.vector.tensor_add(t[:, cs], ps[:, 0:512], pe[:, cs])

        def kv_kproj_neg(hs):
            tn = kpp.tile([128, 1024], F32R, name=f"kp{hs}in", tag=f"kp{hs}in")
            kp[(hs, "in")] = tn
            nc.vector.tensor_scalar_mul(tn[:], kp[(hs, "i")][:], -1.0)

        def kv_kproj(hs, kt):
            for chh in range(2):
                for part in ("r", "i"):
                    kv_kproj_chunk(hs, kt, part, chh)
            kv_kproj_neg(hs)

        # ---- v projection for a head slot ----
        vp = {}   # (hs, chunk) -> [k=128, 258] bf16  ([vp_r | vp_i | 1])

        def kv_vproj(hs, vt):
            vtr = vt[:, 0:1024]; vti = vt[:, 1024:2048]
            for chh in range(8):
                cs = slice(chh * 128, (chh + 1) * 128)
                ps = avp.tile([128, 258], F32, name="vps", tag="av")
                nc.tensor.matmul(ps[:, 0:256], vtr[:, cs], W["wv1"],
                                 start=True, stop=False)
                nc.tensor.matmul(ps[:, 0:256], vti[:, cs], W["wv2"],
                                 start=False, stop=True)
                vt_ = vpp.tile([128, 258], BF16, name=f"vp{hs}_{chh}",
                               tag=f"vp{hs}_{chh}")
                vp[(hs, chh)] = vt_
                if chh % 2 == 0:
                    nc.vector.tensor_copy(vt_[:, 0:256], ps[:, 0:256])
                else:
                    nc.scalar.copy(vt_[:, 0:256], ps[:, 0:256])
                nc.vector.memset(vt_[:, 256:258], 1.0)

        # ---- q projection chunk (chunk == batch) ----
        qp = {}
        for half in (0, 1):
            for part in ("r", "i"):
                qp[(half, part)] = qpp.tile([128, 1536], F32R,
                                            name=f"qp{half}{part}",
                                            tag=f"qp{half}{part}")

        def emit_qproj(ch):
            cs = slice(ch * 512, (ch + 1) * 512)
            for half in (0, 1):
                hs_ = slice(half * 128, (half + 1) * 128)
                for part in ("r", "i"):
                    t = qp[(half, part)]
                    ps = scp.tile([128, 1024], F32, name="scps", tag="sc")
                    if part == "r":
                        nc.tensor.matmul(ps[:, 0:512], W["wq_r"][:, hs_],
                                         qsl(ch, 0), start=True, stop=False)
                        nc.tensor.matmul(ps[:, 0:512], W["wq_in"][:, hs_],
                                         qsl(ch, 1), start=False, stop=True)
                        pe = qsl(ch, 2)
                    else:
                        nc.tensor.matmul(ps[:, 0:512], W["wq_i"][:, hs_],
                                         qsl(ch, 0), start=True, stop=False)
                        nc.tensor.matmul(ps[:, 0:512], W["wq_r"][:, hs_],
                                         qsl(ch, 1), start=False, stop=True)
                        pe = qsl(ch, 3)
                    nc.vector.tensor_add(t[:, cs], ps[:, 0:512], pe)

        # ---- gate projection chunk -> gT [128, 3072] bf16 (r | i) ----
        gT = qpp.tile([128, 3072], BF16, name="gT", tag="gT")
        gT3 = gT[:].rearrange("p (h c) -> p h c", h=2)

        def emit_gproj(ch):
            ps = scp.tile([128, 1024], F32, name="scps", tag="sc")
            nc.tensor.matmul(ps[:, 0:512], W["wg_r"], qsl(ch, 0),
                             start=True, stop=False)
            nc.tensor.matmul(ps[:, 0:512], W["wg_in"], qsl(ch, 1),
                             start=False, stop=True)
            nc.tensor.matmul(ps[:, 512:1024], W["wg_i"], qsl(ch, 0),
                             start=True, stop=False)
            nc.tensor.matmul(ps[:, 512:1024], W["wg_r"], qsl(ch, 1),
                             start=False, stop=True)
            ps3 = ps[:].rearrange("p (h c) -> p h c", h=2)
            nc.scalar.copy(gT3[:, :, ch * 512:(ch + 1) * 512], ps3)

        # =========== score pipeline ===========
        score_ps = {}   # (b, br) -> list of 8 psum tiles
        S2Q = {}        # (b, br) -> 2 s2 tiles [128,2048] f16
        et = {}         # (b, br, p) -> [128,2048] bf16

        def emit_scores(b, br):
            hs = 0 if b < 2 else 1
            qs = slice(b * 512, (b + 1) * 512)
            tiles = []
            for chh in range(8):
                cs = slice(chh * 128, (chh + 1) * 128)
                ps = scp.tile([128, 1024], F32, name="scps", tag="sc")
                nc.tensor.matmul(ps[:, 0:512], kp[(hs, "r")][:, cs],
                                 qp[(br, "r")][:, qs], start=True, stop=False)
                nc.tensor.matmul(ps[:, 0:512], kp[(hs, "i")][:, cs],
                                 qp[(br, "i")][:, qs], start=False, stop=True)
                nc.tensor.matmul(ps[:, 512:1024], kp[(hs, "r")][:, cs],
                                 qp[(br, "i")][:, qs], start=True, stop=False)
                nc.tensor.matmul(ps[:, 512:1024], kp[(hs, "in")][:, cs],
                                 qp[(br, "r")][:, qs], start=False, stop=True)
                tiles.append(ps)
            score_ps[(b, br)] = tiles

        def emit_sq(b, br):
            # squares of a chunk pair land in one sqp tile [128,2048]:
            # chunk 2j: sr->[0:512], si->[1024:1536]; 2j+1: sr->[512:1024],
            # si->[1536:2048]; one DVE f16 2x add [128,1024] per pair.
            mode = SQMODE_EVEN if br == 0 else SQMODE_ODD
            s2_tiles = [hot.tile([128, 2048], F16, name="s2", tag="s2",
                                 bufs=4) for _ in range(2)]
            for j in range(4):
                sqp = hot.tile([128, 2048], F16, name="sqp", tag="sqp",
                               bufs=3)
                sqp3 = sqp[:].rearrange("p (h c) -> p h c", h=2)
                for jj in (0, 1):
                    chh = 2 * j + jj
                    ps = score_ps[(b, br)][chh]
                    ps3 = ps[:].rearrange("p (h c) -> p h c", h=2)
                    dst = sqp3[:, :, jj * 512:(jj + 1) * 512]
                    if mode[chh] == 'S':
                        nc.scalar.activation(dst, ps3, AF.Square, scale=0.5)
                    else:
                        cp = hot.tile([128, 1024], F16, name="cp", tag="cp",
                                      bufs=3)
                        nc.vector.tensor_scalar_mul(cp[:], ps[:], 0.5)
                        cp3 = cp[:].rearrange("p (h c) -> p h c", h=2)
                        nc.vector.tensor_mul(dst, cp3, cp3)
                s2 = s2_tiles[j // 2]
                slot = slice((2 * j % 4) * 512, (2 * j % 4 + 2) * 512)
                nc.vector.tensor_add(s2[:, slot], sqp[:, 0:1024],
                                     sqp[:, 1024:2048])
            score_ps.pop((b, br))
            S2Q[(b, br)] = s2_tiles

        # The tile scheduler reorders engine queues by data deps only, so
        # ACT-table grouping must be enforced with tiny fake-dep biases:
        # all sqrts of batch b wait on the previous exp block (z8), rms
        # rides after the last sqrt (z5), and all exps wait on rms (ze).
        def mkbias(src, val, tag):
            z = epi.tile([128, 1], F32, name=tag, tag=tag, bufs=2)
            nc.vector.tensor_scalar(z[:], src, 0.0, val,
                                    op0=OP.mult, op1=OP.add)
            return z

        def emit_sqrt(b, br, bias):
            for p in range(2):
                nc.scalar.activation(S2Q[(b, br)][p][:], S2Q[(b, br)][p][:],
                                     AF.Sqrt, bias=bias)

        def emit_exp(b, br, bias):
            for p in range(2):
                e = hot.tile([128, 2048], BF16, name="et", tag="et", bufs=6)
                nc.scalar.activation(e[:], S2Q[(b, br)][p][:], AF.Exp,
                                     scale=2.0 * SCALE, bias=bias)
                et[(b, br, p)] = e
            S2Q.pop((b, br))

        # =========== av + epilogue ===========
        EP = {}

        def emit_av_pass(b, br, p, d):
            # one uu-pair pass of one branch
            hs = 0 if b < 2 else 1
            uus = (2 * p, 2 * p + 1)
            avs = {}
            for uu in uus:
                avs[uu] = avp.tile([128, 258], F32, name=f"av{uu}", tag="av")
            for c in range(8):
                e = et[(b, br, c // 4)]
                base = (c % 4) * 512
                for uu in uus:
                    nc.tensor.matmul(
                        avs[uu][:],
                        e[:, base + uu * 128: base + (uu + 1) * 128],
                        vp[(hs, c)][:], start=(c == 0), stop=(c == 7))
            s, inv = (d["s1"], d["inv1"]) if br == 0 else (d["s2g"], d["inv2"])
            for uu in uus:
                nc.vector.tensor_copy(s[:, uu:uu + 1], avs[uu][:, 256:257])
            nc.vector.reciprocal(inv[:, 2 * p:2 * p + 2],
                                 s[:, 2 * p:2 * p + 2])
            if br == 0:
                for uu in uus:
                    nc.vector.tensor_scalar_mul(d["anp"][uu][:],
                                                avs[uu][:, 0:256],
                                                inv[:, uu:uu + 1])
            else:
                nc.vector.tensor_scalar_mul(d["s2v"][:, 2 * p:2 * p + 2],
                                            d["inv2"][:, 2 * p:2 * p + 2],
                                            lam_t[:, 0:1])
                for uu in uus:
                    # t = av2 * (-lam*inv2) + a1n
                    nc.vector.scalar_tensor_tensor(
                        d["t"][uu][:], avs[uu][:, 0:256],
                        d["s2v"][:, uu:uu + 1], d["anp"][uu][:],
                        op0=OP.mult, op1=OP.add)
                    # ss1_u = sum(a1n^2)
                    scr1 = epi.tile([128, 256], F32, name="scr1", tag="scr1",
                                    bufs=2)
                    nc.vector.scalar_tensor_tensor(
                        scr1[:], d["anp"][uu][:], 1.0, d["anp"][uu][:],
                        op0=OP.mult, op1=OP.mult,
                        accum_out=d["ss1"][:, uu:uu + 1])
                    # ss2_u = sum((lam*a2n)^2) = sum((a1n - t)^2);
                    # scaled by 1/lam^2 in the combine
                    dtl = epi.tile([128, 256], F32, name="dtl", tag="dtl",
                                   bufs=2)
                    nc.vector.tensor_sub(dtl[:], d["anp"][uu][:],
                                         d["t"][uu][:])
                    scr2 = epi.tile([128, 256], F32, name="scr2", tag="scr2",
                                    bufs=2)
                    nc.vector.scalar_tensor_tensor(
                        scr2[:], dtl[:], 1.0, dtl[:],
                        op0=OP.mult, op1=OP.mult,
                        accum_out=d["ss2"][:, uu:uu + 1])
                # per-pair combine: ss = ss1 + ss2/lam^2
                cs = slice(2 * p, 2 * p + 2)
                nc.vector.tensor_scalar_mul(d["i2"][:, cs], d["ss2"][:, cs],
                                            l2i_t[:, 0:1])
                nc.vector.tensor_add(d["ss"][:, cs], d["i2"][:, cs],
                                     d["ss1"][:, cs])

        def av_state(b):
            d = {}
            EP[b] = d
            d["anp"] = [epi.tile([128, 256], F32, name="anp", tag="anp",
                                 bufs=4) for _ in range(4)]
            d["t"] = [epi.tile([128, 256], F32, name="tt", tag="tt", bufs=4)
                      for _ in range(4)]
            for nm in ("s1", "inv1", "s2g", "inv2", "s2v", "ss1", "ss2",
                       "ss", "i2"):
                d[nm] = epi.tile([128, 4], F32, name=nm, tag=nm)
            return d

        def emit_av_batch(b):
            d = av_state(b)
            for br in (0, 1):
                for p in (0, 1):
                    emit_av_pass(b, br, p, d)

        def emit_rms(b, p, bias=None):
            d = EP[b]
            if p == 0:
                d["rms"] = epi.tile([128, 4], F32, name="rms", tag="rms")
                d["rinv"] = epi.tile([128, 4], F32, name="rinv", tag="rinv")
            cs = slice(2 * p, 2 * p + 2)
            nc.scalar.activation(d["rms"][:, cs], d["ss"][:, cs], AF.Sqrt,
                                 bias=bias if bias is not None else eps5[:],
                                 scale=1.0 / 256.0)

        def emit_ep_pair(b, pi):
            # rinv, transpose, bf16 gating at DVE 2x, out-proj, out copies
            d = EP[b]
            cs = slice(2 * pi, 2 * pi + 2)
            nc.vector.reciprocal(d["rinv"][:, cs], d["rms"][:, cs])
            tp = scp.tile([128, 1024], F32, name="scps", tag="sc")
            for j in range(2):
                u = 2 * pi + j
                nc.tensor.transpose(tp[:, j * 128:(j + 1) * 128],
                                    d["t"][u][:, 0:128], ident[:])
                nc.tensor.transpose(tp[:, 256 + j * 128: 256 + (j + 1) * 128],
                                    d["t"][u][:, 128:256], ident[:])
            tps = epi.tile([128, 512], BF16, name="tps", tag="tps", bufs=2)
            nc.vector.tensor_copy(tps[:], tp[:, 0:512])
            gr = gT[:, b * 512 + pi * 256: b * 512 + (pi + 1) * 256]
            gi = gT[:, 1536 + b * 512 + pi * 256:
                    1536 + b * 512 + (pi + 1) * 256]
            m1 = epi.tile([128, 256], BF16, name="m1", tag="m1", bufs=2)
            m2 = epi.tile([128, 256], BF16, name="m2", tag="m2", bufs=2)
            m3 = epi.tile([128, 256], BF16, name="m3", tag="m3", bufs=2)
            m4 = epi.tile([128, 256], BF16, name="m4", tag="m4", bufs=2)
            nc.vector.tensor_mul(m1[:], gr, tps[:, 0:256])
            nc.vector.tensor_mul(m2[:], gi, tps[:, 256:512])
            nc.vector.tensor_mul(m3[:], gi, tps[:, 0:256])
            nc.vector.tensor_mul(m4[:], gr, tps[:, 256:512])
            cr = epi.tile([128, 256], BF16, name="cr", tag="cr", bufs=2)
            ci = epi.tile([128, 256], BF16, name="ci", tag="ci", bufs=2)
            nc.gpsimd.tensor_sub(cr[:], m1[:], m2[:])
            nc.gpsimd.tensor_add(ci[:], m3[:], m4[:])
            po = scp.tile([128, 1024], F32, name="scps", tag="sc")
            for j in range(2):
                pos = slice(j * 256, (j + 1) * 256)
                nc.tensor.matmul(po[:, pos], cr[:, j * 128:(j + 1) * 128],
                                 wo1b[:], start=True, stop=False)
                nc.tensor.matmul(po[:, pos], ci[:, j * 128:(j + 1) * 128],
                                 wo2b[:], start=False, stop=True)
            for j in range(2):
                u = 2 * pi + j
                iu = b * 4 + u
                ot = osb.tile([128, 256], F32, name="ot", tag="ot")
                nc.scalar.mul(ot[:], po[:, j * 256:(j + 1) * 256],
                              d["rinv"][:, u:u + 1])
                nc.sync.dma_start(out_d[iu * 128:(iu + 1) * 128, :], ot[:])

        # ---- emission sequence ----
        # prep: kproj chunk 0 asap, then drains, qproj ch0, scores
        for part in ("r", "i"):
            kv_kproj_chunk(0, kin_t, part, 0)
        for part in ("r", "i"):
            kv_kproj_chunk(0, kin_t, part, 1)
        kv_kproj_neg(0)
        emit_qproj(0)
        emit_scores(0, 0); emit_sq(0, 0)
        emit_sqrt(0, 0, eps8[:])
        emit_scores(0, 1); emit_sq(0, 1)
        emit_sqrt(0, 1, eps8[:])
        ze = mkbias(S2Q[(0, 1)][1][:, 0:1], 0.0, "ze")
        emit_exp(0, 0, ze[:]); emit_exp(0, 1, ze[:])
        emit_gproj(0)
        kv_vproj(0, vin_t)
        emit_qproj(1)

        # head slot 1 k-side DMAs (sync: before any data-dependent issues)
        kin1 = kin.tile([128, 4096], F32R, name="kin1", tag="kin0")
        nc.sync.dma_start(kin1[:, 0:2048], kpack[:, 4096:6144])
        nc.sync.dma_start(kin1[:, 2048:4096], kpack[:, 6144:8192])
        vin1 = kin.tile([128, 2048], F32R, name="vin1", tag="vin0")
        nc.gpsimd.dma_start(vin1[:], vpack[:, 2048:4096])

        emit_scores(1, 0); emit_sq(1, 0)
        z8 = mkbias(et[(0, 1, 1)][:, 0:1], 1e-8, "z8")
        emit_sqrt(1, 0, z8[:])
        emit_av_batch(0)
        emit_scores(1, 1); emit_sq(1, 1)
        emit_sqrt(1, 1, z8[:])
        z5 = mkbias(S2Q[(1, 1)][1][:, 0:1], 1e-5, "z5")
        emit_rms(0, 0, z5[:]); emit_rms(0, 1, z5[:])
        ze = mkbias(EP[0]["rms"][:, 0:1], 0.0, "ze")
        emit_exp(1, 0, ze[:]); emit_exp(1, 1, ze[:])
        emit_ep_pair(0, 0); emit_ep_pair(0, 1)

        kv_kproj(1, kin1)
        kv_vproj(1, vin1)
        emit_gproj(1)
        emit_qproj(2)
        emit_gproj(2)
        qinctx.close()

        emit_scores(2, 0); emit_sq(2, 0)
        z8 = mkbias(et[(1, 1, 1)][:, 0:1], 1e-8, "z8")
        emit_sqrt(2, 0, z8[:])
        emit_av_batch(1)
        emit_scores(2, 1); emit_sq(2, 1)
        emit_sqrt(2, 1, z8[:])
        z5 = mkbias(S2Q[(2, 1)][1][:, 0:1], 1e-5, "z5")
        emit_rms(1, 0, z5[:]); emit_rms(1, 1, z5[:])
        ze = mkbias(EP[1]["rms"][:, 0:1], 0.0, "ze")
        emit_exp(2, 0, ze[:]); emit_exp(2, 1, ze[:])
        emit_ep_pair(1, 0); emit_ep_pair(1, 1)
        kinctx.close()

        # tail: pair-major av(2) + epilogue interleave
        d2 = av_state(2)
        emit_av_pass(2, 0, 0, d2)
        emit_av_pass(2, 1, 0, d2)
        emit_rms(2, 0)
        emit_ep_pair(2, 0)
        emit_av_pass(2, 0, 1, d2)
        emit_av_pass(2, 1, 1, d2)
        emit_rms(2, 1)
        emit_ep_pair(2, 1)

    nc.compile()
    return nc


def _get_program():
    if "nc" not in _prog_cache:
        _prog_cache["nc"] = _build_program()
    return _prog_cache["nc"]


def _prep_inputs(inputs):
    f = {k: np.asarray(v, dtype=np.float32) for k, v in inputs.items()}
    lam1 = np.float32(np.exp(np.float32(np.sum(f["lq1"] * f["lk1"]))))
    lam2 = np.float32(np.exp(np.float32(np.sum(f["lq2"] * f["lk2"]))))
    x = np.float32(lam1 - lam2 + np.float32(LAMBDA_INIT))
    lam = np.float32(1.0 / (1.0 + np.exp(-x)))

    wq_rT = f["qw_r"].T          # [128, 256]
    wq_iT = f["qw_i"].T
    wk_rT = f["kw_r"].T          # [128, 128]
    wk_iT = f["kw_i"].T
    vw_rT = f["vw_r"].T; vw_iT = f["vw_i"].T
    wv1 = np.concatenate([vw_rT, vw_iT], 1)
    wv2 = np.concatenate([-vw_iT, vw_rT], 1)
    wg_rT = f["gw_r"].T; wg_iT = f["gw_i"].T
    ow_rT = f["ow_r"].T; ow_iT = f["ow_i"].T
    wo1 = np.concatenate([ow_rT, ow_iT], 1)
    wo2 = np.concatenate([-ow_iT, ow_rT], 1)
    wmap = {"wk_r": wk_rT, "wk_i": wk_iT, "wk_in": -wk_iT,
            "wq_r": wq_rT, "wq_i": wq_iT, "wq_in": -wq_iT,
            "wv1": wv1, "wv2": wv2,
            "wg_r": wg_rT, "wg_i": wg_iT, "wg_in": -wg_iT,
            "wo1": wo1, "wo2": wo2}
    wall = np.zeros((128, WALLW), np.float32)
    for nm, (c, w) in WCOLS.items():
        wall[:, c:c + w] = wmap[nm]
    shared = {
        "wall": wall,
        "lamneg": np.full((128, 1), -lam, np.float32),
        "lam2inv": np.full((128, 1), 1.0 / (lam * lam), np.float32),
    }

    in_maps = []
    for c in range(NCORES):
        units = _core_units(c)
        heads = [units[0][0], units[8][0]]
        m = dict(shared)

        def pack_q(t):
            cols = [t[0, h, q * 128:(q + 1) * 128, :].T for (h, q) in units]
            return np.concatenate(cols, 1)
        qtr = pack_q(f["q_r"]); qti = pack_q(f["q_i"])
        pqr = pack_q(f["pe_q_r"]); pqi = pack_q(f["pe_q_i"])
        m["qpack"] = np.ascontiguousarray(np.concatenate(
            [np.concatenate([qtr[:, ch * 512:(ch + 1) * 512],
                             qti[:, ch * 512:(ch + 1) * 512],
                             pqr[:, ch * 512:(ch + 1) * 512],
                             pqi[:, ch * 512:(ch + 1) * 512]], 1)
             for ch in range(3)], 1))

        kk = []
        vv = []
        for h in heads:
            kk.append(np.concatenate(
                [f["k_r"][0, h].T, f["k_i"][0, h].T,
                 f["pe_k_r"][0, h].T, f["pe_k_i"][0, h].T], 1))
            vv.append(np.concatenate([f["v_r"][0, h].T, f["v_i"][0, h].T], 1))
        m["kpack"] = np.ascontiguousarray(np.concatenate(kk, 1))
        m["vpack"] = np.ascontiguousarray(np.concatenate(vv, 1))
        in_maps.append(m)
    return in_maps


def _unpack(results):
    out_r = np.zeros((1, H, S, D), np.float32)
    out_i = np.zeros((1, H, S, D), np.float32)
    for c in range(NCORES):
        o = results[c]["out"]
        for u, (h, q) in enumerate(_core_units(c)):
            blk = o[u * 128:(u + 1) * 128]
            out_r[0, h, q * 128:(q + 1) * 128, :] = blk[:, 0:128]
            out_i[0, h, q * 128:(q + 1) * 128, :] = blk[:, 128:256]
    return out_r, out_i


def _run(inputs, trace=False, tmpdir=None):
    nc = _get_program()
    in_maps = _prep_inputs(inputs)
    res = run_bass_kernel_spmd(nc, in_maps, list(range(NCORES)), trace=trace,
                               tmpdir=tmpdir)
    return _unpack(res.results), res


def kernel(**inputs):
    (out_r, out_i), _ = _run(inputs, trace=False)
    return out_r, out_i
